# revision 11
# baseline (speedup 1.0000x reference)
# Trainium2 Bass kernel for nn_ExpertLinear (MoE grouped GEMM with routing).
#
# Strategy: data-parallel over tokens (8 cores), full weights replicated,
# fp16 compute with fp32 PSUM accumulation (measured rel err ~3.6e-4).
# Per core:
#   1. dma_gather(transpose=True) pulls the core's token rows from HBM x
#      (fp16) directly into the transposed [d_in, tokens] stationary-operand
#      layout, grouped by expert (per-expert groups padded to 128-row tiles,
#      tile counts shared across cores so one NEFF serves all 8).
#   2. Grouped GEMM per expert row-tile: 8 k-tile matmuls accumulate into
#      PSUM; eviction applies the per-row gate (DVE tensor_scalar) while
#      casting into one of TWO fp16 y buffers (first/second half of the
#      expert processing order).
#   3. Combine: tokens are classed AA/AB/BB by which half their two rows
#      live in. AA (and the A side of AB) gathers+adds run DURING the
#      second half of the GEMM; only the B-side work remains as a tail.
#      SBUF-source dma_gather returns each token's rows transposed; DVE
#      adds produce out^T chunks written to DRAM. Host de-transposes and
#      scatters rows by the token->core/column assignment.
import os
import numpy as np

import concourse.bacc as bacc
import concourse.bass as bass
import concourse.mybir as mybir
import concourse.tile as tile
from concourse.bass_utils import run_bass_kernel_spmd

N_TOK = 8192
TOPK = 2
N_EXP = 8
D_IN = 1024
D_OUT = 1024
NCORES = 8
TPC = N_TOK // NCORES          # tokens per core
P = 128
KTILES = D_IN // P             # 8 k-tiles over d_in
F16 = mybir.dt.float16
F32 = mybir.dt.float32
I16 = mybir.dt.int16
NHALF = N_EXP // 2             # experts per y-buffer half


def _pack16(flat):
    # [16, n/16] block (idx j at [j%16, j//16]), replicated into all eight
    # 16-partition groups — each GpSimd Q7 core reads its own copy.
    return np.ascontiguousarray(np.tile(flat.reshape(-1, 16).T, (8, 1)))


def _pad128(n):
    return max(P, -(-int(n) // P) * P)


def _plan(tok, sei, g_row):
    """Host routing plan. Returns per-core tables and shared shape params."""
    # experts of each token (k rows per token)
    order_by_tok = np.argsort(tok, kind="stable")
    te = sei[order_by_tok].reshape(N_TOK, TOPK)
    pair_id = te[:, 0] * N_EXP + te[:, 1]

    # balanced token->core assignment: round-robin inside expert-pair groups
    order = np.argsort(pair_id, kind="stable")
    core_of_token = np.empty(N_TOK, np.int64)
    core_of_token[order] = np.arange(N_TOK) % NCORES

    core_of_row = core_of_token[tok]
    rows_per_core = [np.where(core_of_row == c)[0] for c in range(NCORES)]
    cnt = np.zeros((NCORES, N_EXP), np.int64)
    for c in range(NCORES):
        cnt[c] = np.bincount(sei[rows_per_core[c]], minlength=N_EXP)

    # process experts smallest-first (fast first gather; halves balanced)
    perm = np.argsort(cnt.max(axis=0), kind="stable")
    T = np.maximum(1, -(-cnt.max(axis=0)[perm] // P))   # tiles per slot
    off = np.concatenate([[0], np.cumsum(T) * P])        # padded row offsets
    NP = int(off[-1])
    slot_of_expert = np.empty(N_EXP, np.int64)
    slot_of_expert[perm] = np.arange(N_EXP)
    rowsA = int(off[NHALF])                              # rows in first half

    # token classes per core: 0=AA (both rows in first half), 1=AB, 2=BB
    half_of_expert = (slot_of_expert >= NHALF).astype(np.int64)
    th = half_of_expert[te]                              # [N_TOK, 2]
    cls_of_token = th.sum(axis=1)

    csize = np.zeros((NCORES, 3), np.int64)
    for c in range(NCORES):
        for cl in range(3):
            csize[c, cl] = np.count_nonzero(
                (core_of_token == c) & (cls_of_token == cl))
    CP = [_pad128(csize[:, cl].max()) for cl in range(3)]  # padded class cols
    TPCP = sum(CP)
    cbase = [0, CP[0], CP[0] + CP[1]]

    per_core = []
    for c in range(NCORES):
        rows_c = rows_per_core[c]
        s_c = slot_of_expert[sei[rows_c]]
        ordr = np.argsort(s_c, kind="stable")            # slot-major rows
        rows_c = rows_c[ordr]
        s_c = s_c[ordr]
        scnt = np.bincount(s_c, minlength=N_EXP)
        within = np.arange(rows_c.size) - np.concatenate(
            [[0], np.cumsum(scnt)])[s_c]
        loc = off[s_c] + within                          # padded row slot

        gidx_flat = np.zeros(NP, np.int16)
        grow_flat = np.zeros(NP, np.float32)
        gidx_flat[loc] = tok[rows_c].astype(np.int16)
        grow_flat[loc] = g_row[rows_c]

        # per-token: the two row locations + their halves
        tloc = {}
        for j in range(rows_c.size):
            tloc.setdefault(int(tok[rows_c[j]]), []).append(
                (int(loc[j]), int(loc[j]) >= rowsA))

        # order tokens: class AA | AB | BB, each padded to CP[cl]
        toks_c = np.where(core_of_token == c)[0]
        col_tokens = np.full(TPCP, -1, np.int64)
        idx0 = np.zeros(TPCP, np.int16)                 # A-side (or first)
        idx1 = np.zeros(TPCP, np.int16)                 # B-side (or second)
        ptr = [0, 0, 0]
        for t in toks_c:
            cl = int(cls_of_token[t])
            col = cbase[cl] + ptr[cl]
            ptr[cl] += 1
            col_tokens[col] = t
            (l0, h0), (l1, h1) = tloc[int(t)]
            if cl == 1 and h0:                          # A side first
                l0, l1 = l1, l0
            idx0[col] = l0 if cl == 0 or cl == 1 else l0 - rowsA
            idx1[col] = (l1 - rowsA) if cl >= 1 else l1
        per_core.append(
            dict(
                gidx=_pack16(gidx_flat),
                grow=np.ascontiguousarray(grow_flat.reshape(-1, P).T),
                r0i=_pack16(idx0),
                r1i=_pack16(idx1),
                _col_tokens=col_tokens,
            )
        )
    return T, NP, rowsA, CP, per_core, perm


def _build_nc(T, CP):
    NP = int(T.sum()) * P
    NB = NP // P
    off = np.concatenate([[0], np.cumsum(T)]) * P
    rowsA = int(off[NHALF])
    NBA = rowsA // P
    NBB = NB - NBA
    TPCP = sum(CP)
    cbase = [0, CP[0], CP[0] + CP[1]]

    nc = bacc.Bacc("TRN2", target_bir_lowering=False, debug=False,
                   num_devices=NCORES)

    xh = nc.dram_tensor("xh", [N_TOK, D_IN], F16, kind="ExternalInput")
    wh = nc.dram_tensor("wh", [N_EXP, P, KTILES, D_OUT], F16,
                        kind="ExternalInput")
    gidx = nc.dram_tensor("gidx", [P, NP // 16], I16, kind="ExternalInput")
    grow = nc.dram_tensor("grow", [P, NB], F32, kind="ExternalInput")
    r0i = nc.dram_tensor("r0i", [P, TPCP // 16], I16, kind="ExternalInput")
    r1i = nc.dram_tensor("r1i", [P, TPCP // 16], I16, kind="ExternalInput")
    outT = nc.dram_tensor("outT", [P, D_OUT // P, TPCP], F32,
                          kind="ExternalOutput")

    with tile.TileContext(nc) as tc:
        with (
            tc.tile_pool(name="const", bufs=1) as kpool,
            tc.tile_pool(name="w", bufs=2) as wpool,
            tc.tile_pool(name="xT", bufs=4) as xpool,
            tc.tile_pool(name="y", bufs=1) as ypool,
            tc.tile_pool(name="cmb", bufs=2) as cpool,
            tc.tile_pool(name="ot", bufs=2) as opool,
            tc.tile_pool(name="ps", bufs=4, space="PSUM") as ppool,
        ):
            # tiny dummy gather: forces the Q7 extended-instruction library
            # load to overlap the startup DMAs instead of the first real
            # gather.
            warm_i = kpool.tile([P, 8], I16)
            nc.gpsimd.memset(warm_i[:], 0)
            warm_o = kpool.tile([P, 1, P], F16)
            nc.gpsimd.dma_gather(warm_o[:], xh[:].rearrange("n (a b) -> (n a) b", b=P),
                                 warm_i[:], num_idxs=P,
                                 num_idxs_reg=P, elem_size=P,
                                 transpose=True)

            gidx_t = kpool.tile([P, NP // 16], I16)
            nc.gpsimd.dma_start(gidx_t[:], gidx[:])

            # dispatch gathers (expert 0 split per row-tile so the first
            # matmul can start early)
            x_tiles = []
            for e in range(N_EXP):
                ne = int(T[e]) * P
                if e == 0:
                    # split per row-tile so the first matmul starts early
                    parts = []
                    for t in range(int(T[e])):
                        xp = xpool.tile([P, KTILES, P], F16, tag=f"x0_{t}")
                        nc.gpsimd.dma_gather(
                            xp[:], xh[:],
                            gidx_t[:, (off[e] + t * P) // 16:
                                   (off[e] + (t + 1) * P) // 16],
                            num_idxs=P, num_idxs_reg=P, elem_size=D_IN,
                            transpose=True, single_packet=False,
                        )
                        parts.append(xp)
                    x_tiles.append(parts)
                else:
                    x_t = xpool.tile([P, KTILES, ne], F16, tag="xT")
                    nc.gpsimd.dma_gather(
                        x_t[:], xh[:],
                        gidx_t[:, off[e] // 16:(off[e] + ne) // 16],
                        num_idxs=ne, num_idxs_reg=ne, elem_size=D_IN,
                        transpose=True, single_packet=False,
                    )
                    x_tiles.append(x_t)

            grow_t = kpool.tile([P, NB], F32)
            nc.sync.dma_start(grow_t[:], grow[:])
            r0_t = kpool.tile([P, TPCP // 16], I16)
            nc.sync.dma_start(r0_t[:], r0i[:])
            r1_t = kpool.tile([P, TPCP // 16], I16)
            nc.sync.dma_start(r1_t[:], r1i[:])

            yA = ypool.tile([P, NBA, D_OUT], F16)
            yB = ypool.tile([P, NBB, D_OUT], F16)

            def gemm_expert(e):
                w_t = wpool.tile([P, KTILES, D_OUT], F16, tag="w")
                for kk in range(KTILES):
                    nc.scalar.dma_start(w_t[:, kk], wh[e, :, kk])
                x_t = x_tiles[e]
                for t in range(int(T[e])):
                    rt_g = off[e] // P + t
                    ps0 = ppool.tile([P, 512], F32, tag="ps")
                    ps1 = ppool.tile([P, 512], F32, tag="ps")
                    for kk in range(KTILES):
                        if e == 0:
                            lhsT = x_t[t][:, kk, :]
                        else:
                            lhsT = x_t[:, kk, t * P:(t + 1) * P]
                        nc.tensor.matmul(ps0[:], lhsT, w_t[:, kk, 0:512],
                                         start=(kk == 0),
                                         stop=(kk == KTILES - 1))
                        nc.tensor.matmul(ps1[:], lhsT, w_t[:, kk, 512:1024],
                                         start=(kk == 0),
                                         stop=(kk == KTILES - 1))
                    if rt_g < NBA:
                        ydst, yrow = yA, rt_g
                    else:
                        ydst, yrow = yB, rt_g - NBA
                    gsc = grow_t[:, rt_g:rt_g + 1]
                    nc.vector.tensor_scalar_mul(ydst[:, yrow, 0:512],
                                                ps0[:], gsc)
                    nc.vector.tensor_scalar_mul(ydst[:, yrow, 512:1024],
                                                ps1[:], gsc)

            def combine(cl):
                # class cl columns, chunked by 256
                base, width = cbase[cl], CP[cl]
                src0 = yA if cl <= 1 else yB
                nr0 = NBA if cl <= 1 else NBB
                src1 = yA if cl == 0 else yB
                nr1 = NBA if cl == 0 else NBB
                for s in range(0, width, 256):
                    ch = min(256, width - s)
                    g0 = cpool.tile([P, D_OUT // P, ch], F16, tag="c0")
                    g1 = cpool.tile([P, D_OUT // P, ch], F16, tag="c1")
                    for dst, ridx, src, nr in ((g0, r0_t, src0, nr0),
                                               (g1, r1_t, src1, nr1)):
                        nc.gpsimd.dma_gather(
                            dst[:], src[:],
                            ridx[:, (base + s) // 16:(base + s + ch) // 16],
                            num_idxs=ch, num_idxs_reg=ch, elem_size=D_OUT,
                            transpose=True, single_packet=False,
                            sbuf_tokens_per_rank=P,
                            sbuf_free_dim_per_rank=D_OUT * 2,
                        )
                    ot = opool.tile([P, D_OUT // P, ch], F32, tag="ot")
                    nc.vector.tensor_add(out=ot[:], in0=g0[:], in1=g1[:])
                    nc.sync.dma_start(
                        outT[:, :, base + s:base + s + ch], ot[:])

            for e in range(NHALF):
                gemm_expert(e)
            combine(0)                    # AA: overlaps second-half GEMM
            for e in range(NHALF, N_EXP):
                gemm_expert(e)
            combine(1)                    # AB tail (A side could overlap)
            combine(2)                    # BB tail

    nc.compile()
    return nc


def _prep(inputs):
    x = np.asarray(inputs["input"], np.float32)
    w = np.asarray(inputs["weight"], np.float32)
    k = int(np.asarray(inputs["k"]))
    assert k == TOPK
    sei = np.asarray(inputs["sorted_expert_indices"]).astype(np.int64)
    ssi = np.asarray(inputs["sorted_scattered_indices"]).astype(np.int64)
    gates = np.asarray(inputs["gates"], np.float32)

    tok = ssi // k
    g_row = gates.reshape(-1)[ssi]

    T, NP, rowsA, CP, per_core, perm = _plan(tok, sei, g_row)

    xh = x.astype(np.float16)
    whp = np.ascontiguousarray(
        w.reshape(N_EXP, KTILES, P, D_OUT).transpose(0, 2, 1, 3)
    ).astype(np.float16)[perm]            # expert slot order

    in_maps, col_tokens = [], []
    for c in range(NCORES):
        m = {k2: v for k2, v in per_core[c].items() if not k2.startswith("_")}
        m["xh"] = xh
        m["wh"] = np.ascontiguousarray(whp)
        in_maps.append(m)
        col_tokens.append(per_core[c]["_col_tokens"])
    return T, CP, in_maps, col_tokens


def _run(inputs, trace=False, trace_kwargs=None):
    T, CP, in_maps, col_tokens = _prep(inputs)
    nc = _build_nc(T, CP)
    res = run_bass_kernel_spmd(
        nc, in_maps, core_ids=list(range(NCORES)), trace=trace,
        **(trace_kwargs or {}),
    )
    out = np.zeros((N_TOK, D_OUT), np.float32)
    for c in range(NCORES):
        oT = res.results[c]["outT"]            # [P, D_OUT//P, TPCP]
        shard = oT.transpose(2, 1, 0).reshape(-1, D_OUT)
        valid = col_tokens[c] >= 0
        out[col_tokens[c][valid]] = shard[valid]
    return out, res


def kernel(**inputs) -> np.ndarray:
    out, _ = _run(inputs, trace=bool(int(os.environ.get("KERNEL_TRACE", "0"))))
    return out


# revision 12
# speedup vs baseline: 1.2530x; 1.2530x over previous
# Trainium2 Bass kernel for nn_ExpertLinear (MoE grouped GEMM with routing).
#
# Strategy: data-parallel over tokens (8 cores), full weights replicated,
# fp16 compute with fp32 PSUM accumulation (measured rel err ~3.6e-4).
# Per core:
#   1. dma_gather(transpose=True) pulls the core's token rows from HBM x
#      (fp16) directly into the transposed [d_in, tokens] stationary-operand
#      layout, grouped by expert (per-expert groups padded to 128-row tiles,
#      tile counts shared across cores so one NEFF serves all 8).
#   2. Grouped GEMM per expert row-tile: 8 k-tile matmuls accumulate into
#      PSUM; eviction applies the per-row gate (DVE tensor_scalar) while
#      casting into an fp16 y buffer.
#   3. Combine: SBUF-source dma_gather (transpose mode) pulls each token's
#      two gated y rows (all gathers batched into dedicated tiles), DVE adds
#      produce out^T chunks written to DRAM. Host de-transposes and scatters
#      rows by the token->core assignment.
import os
import numpy as np

import concourse.bacc as bacc
import concourse.bass as bass
import concourse.mybir as mybir
import concourse.tile as tile
from concourse.bass_utils import run_bass_kernel_spmd

N_TOK = 8192
TOPK = 2
N_EXP = 8
D_IN = 1024
D_OUT = 1024
NCORES = 8
TPC = N_TOK // NCORES          # tokens per core
P = 128
KTILES = D_IN // P             # 8 k-tiles over d_in
F16 = mybir.dt.float16
F32 = mybir.dt.float32
I16 = mybir.dt.int16


def _pack16(flat):
    # [16, n/16] block (idx j at [j%16, j//16]), replicated into all eight
    # 16-partition groups — each GpSimd Q7 core reads its own copy.
    return np.ascontiguousarray(np.tile(flat.reshape(-1, 16).T, (8, 1)))


def _plan(tok, sei, g_row):
    """Host routing plan. Returns per-core index/gate tables and the shared
    per-expert tile counts T (max over cores, so one NEFF serves all 8)."""
    order_by_tok = np.argsort(tok, kind="stable")
    te = sei[order_by_tok].reshape(N_TOK, TOPK)
    pair_id = te[:, 0] * N_EXP + te[:, 1]

    # balanced token->core assignment: round-robin inside expert-pair groups
    order = np.argsort(pair_id, kind="stable")
    core_of_token = np.empty(N_TOK, np.int64)
    core_of_token[order] = np.arange(N_TOK) % NCORES

    token_ids = [np.where(core_of_token == c)[0] for c in range(NCORES)]
    token_pos = np.empty(N_TOK, np.int64)
    for c in range(NCORES):
        token_pos[token_ids[c]] = np.arange(TPC)

    core_of_row = core_of_token[tok]
    rows_per_core = [np.where(core_of_row == c)[0] for c in range(NCORES)]
    cnt = np.zeros((NCORES, N_EXP), np.int64)
    for c in range(NCORES):
        cnt[c] = np.bincount(sei[rows_per_core[c]], minlength=N_EXP)

    # process experts smallest-first (fast first gather)
    perm = np.argsort(cnt.max(axis=0), kind="stable")
    T = np.maximum(1, -(-cnt.max(axis=0)[perm] // P))
    off = np.concatenate([[0], np.cumsum(T) * P])
    NP = int(off[-1])
    slot_of_expert = np.empty(N_EXP, np.int64)
    slot_of_expert[perm] = np.arange(N_EXP)

    per_core = []
    for c in range(NCORES):
        rows_c = rows_per_core[c]
        s_c = slot_of_expert[sei[rows_c]]
        ordr = np.argsort(s_c, kind="stable")
        rows_c = rows_c[ordr]
        s_c = s_c[ordr]
        scnt = np.bincount(s_c, minlength=N_EXP)
        within = np.arange(rows_c.size) - np.concatenate(
            [[0], np.cumsum(scnt)])[s_c]
        loc = off[s_c] + within

        gidx_flat = np.zeros(NP, np.int16)
        grow_flat = np.zeros(NP, np.float32)
        gidx_flat[loc] = tok[rows_c].astype(np.int16)
        grow_flat[loc] = g_row[rows_c]

        pos = token_pos[tok[rows_c]]
        r0_flat = np.zeros(TPC, np.int16)
        r1_flat = np.zeros(TPC, np.int16)
        seen = np.zeros(TPC, bool)
        for j in range(rows_c.size):
            p_ = pos[j]
            if seen[p_]:
                r1_flat[p_] = loc[j]
            else:
                r0_flat[p_] = loc[j]
                seen[p_] = True
        assert seen.all()

        per_core.append(
            dict(
                gidx=_pack16(gidx_flat),
                grow=np.ascontiguousarray(grow_flat.reshape(-1, P).T),
                r0i=_pack16(r0_flat),
                r1i=_pack16(r1_flat),
            )
        )
    return T, NP, per_core, token_ids, perm


def _build_nc(T):
    NP = int(T.sum()) * P
    NB = NP // P
    off = np.concatenate([[0], np.cumsum(T)]) * P

    nc = bacc.Bacc("TRN2", target_bir_lowering=False, debug=False,
                   num_devices=NCORES)

    xh = nc.dram_tensor("xh", [N_TOK, D_IN], F16, kind="ExternalInput")
    wh = nc.dram_tensor("wh", [N_EXP, P, KTILES, D_OUT], F16,
                        kind="ExternalInput")
    gidx = nc.dram_tensor("gidx", [P, NP // 16], I16, kind="ExternalInput")
    grow = nc.dram_tensor("grow", [P, NB], F32, kind="ExternalInput")
    r0i = nc.dram_tensor("r0i", [P, TPC // 16], I16, kind="ExternalInput")
    r1i = nc.dram_tensor("r1i", [P, TPC // 16], I16, kind="ExternalInput")
    outT = nc.dram_tensor("outT", [P, D_OUT // P, TPC], F32,
                          kind="ExternalOutput")

    CH = 256
    NCH = TPC // CH

    with tile.TileContext(nc) as tc:
        with (
            tc.tile_pool(name="const", bufs=1) as kpool,
            tc.tile_pool(name="w", bufs=2) as wpool,
            tc.tile_pool(name="xT", bufs=4) as xpool,
            tc.tile_pool(name="y", bufs=1) as ypool,
            tc.tile_pool(name="cmb", bufs=1) as cpool,
            tc.tile_pool(name="ot", bufs=2) as opool,
            tc.tile_pool(name="ps", bufs=4, space="PSUM") as ppool,
        ):
            # tiny dummy gather: pull the Q7 extended-instruction library
            # load into the startup window
            warm_i = kpool.tile([P, 8], I16)
            nc.gpsimd.memset(warm_i[:], 0)
            warm_o = kpool.tile([P, 1, P], F16)
            nc.gpsimd.dma_gather(
                warm_o[:], xh[:].rearrange("n (a b) -> (n a) b", b=P),
                warm_i[:], num_idxs=P, num_idxs_reg=P, elem_size=P,
                transpose=True)

            gidx_t = kpool.tile([P, NP // 16], I16)
            nc.gpsimd.dma_start(gidx_t[:], gidx[:])

            # dispatch gathers up front (expert 0 split per row-tile so the
            # first matmul starts early)
            x_tiles = []
            for e in range(N_EXP):
                ne = int(T[e]) * P
                if e == 0:
                    parts = []
                    for t in range(int(T[e])):
                        xp = xpool.tile([P, KTILES, P], F16, tag=f"x0_{t}")
                        nc.gpsimd.dma_gather(
                            xp[:], xh[:],
                            gidx_t[:, (off[e] + t * P) // 16:
                                   (off[e] + (t + 1) * P) // 16],
                            num_idxs=P, num_idxs_reg=P, elem_size=D_IN,
                            transpose=True,
                        )
                        parts.append(xp)
                    x_tiles.append(parts)
                else:
                    x_t = xpool.tile([P, KTILES, ne], F16, tag="xT")
                    nc.gpsimd.dma_gather(
                        x_t[:], xh[:],
                        gidx_t[:, off[e] // 16:(off[e] + ne) // 16],
                        num_idxs=ne, num_idxs_reg=ne, elem_size=D_IN,
                        transpose=True,
                    )
                    x_tiles.append(x_t)

            grow_t = kpool.tile([P, NB], F32)
            nc.sync.dma_start(grow_t[:], grow[:])
            r0_t = kpool.tile([P, TPC // 16], I16)
            nc.sync.dma_start(r0_t[:], r0i[:])
            r1_t = kpool.tile([P, TPC // 16], I16)
            nc.sync.dma_start(r1_t[:], r1i[:])

            y_t = ypool.tile([P, NB, D_OUT], F16)

            for e in range(N_EXP):
                w_t = wpool.tile([P, KTILES, D_OUT], F16, tag="w")
                for kk in range(KTILES):
                    nc.scalar.dma_start(w_t[:, kk], wh[e, :, kk])
                x_t = x_tiles[e]
                for t in range(int(T[e])):
                    rt_g = off[e] // P + t
                    ps0 = ppool.tile([P, 512], F32, tag="ps")
                    ps1 = ppool.tile([P, 512], F32, tag="ps")
                    for kk in range(KTILES):
                        if e == 0:
                            lhsT = x_t[t][:, kk, :]
                        else:
                            lhsT = x_t[:, kk, t * P:(t + 1) * P]
                        nc.tensor.matmul(ps0[:], lhsT, w_t[:, kk, 0:512],
                                         start=(kk == 0),
                                         stop=(kk == KTILES - 1))
                        nc.tensor.matmul(ps1[:], lhsT, w_t[:, kk, 512:1024],
                                         start=(kk == 0),
                                         stop=(kk == KTILES - 1))
                    gsc = grow_t[:, rt_g:rt_g + 1]
                    nc.vector.tensor_scalar_mul(y_t[:, rt_g, 0:512],
                                                ps0[:], gsc)
                    nc.vector.tensor_scalar_mul(y_t[:, rt_g, 512:1024],
                                                ps1[:], gsc)

            # combine: batch ALL gathers (dedicated tiles), then adds+stores
            gath = []
            for h in range(NCH):
                g0 = cpool.tile([P, D_OUT // P, CH], F16, tag=f"c0_{h}")
                g1 = cpool.tile([P, D_OUT // P, CH], F16, tag=f"c1_{h}")
                for dst, ridx in ((g0, r0_t), (g1, r1_t)):
                    nc.gpsimd.dma_gather(
                        dst[:], y_t[:],
                        ridx[:, h * (CH // 16):(h + 1) * (CH // 16)],
                        num_idxs=CH, num_idxs_reg=CH, elem_size=D_OUT,
                        transpose=True,
                        sbuf_tokens_per_rank=P,
                        sbuf_free_dim_per_rank=D_OUT * 2,
                    )
                gath.append((g0, g1))
            for h, (g0, g1) in enumerate(gath):
                ot = opool.tile([P, D_OUT // P, CH], F32, tag="ot")
                nc.vector.tensor_add(out=ot[:], in0=g0[:], in1=g1[:])
                nc.sync.dma_start(outT[:, :, h * CH:(h + 1) * CH], ot[:])

    nc.compile()
    return nc


def _prep(inputs):
    x = np.asarray(inputs["input"], np.float32)
    w = np.asarray(inputs["weight"], np.float32)
    k = int(np.asarray(inputs["k"]))
    assert k == TOPK
    sei = np.asarray(inputs["sorted_expert_indices"]).astype(np.int64)
    ssi = np.asarray(inputs["sorted_scattered_indices"]).astype(np.int64)
    gates = np.asarray(inputs["gates"], np.float32)

    tok = ssi // k
    g_row = gates.reshape(-1)[ssi]

    T, NP, per_core, token_ids, perm = _plan(tok, sei, g_row)

    xh = x.astype(np.float16)
    whp = np.ascontiguousarray(
        w.reshape(N_EXP, KTILES, P, D_OUT).transpose(0, 2, 1, 3)
    ).astype(np.float16)[perm]

    in_maps = []
    for c in range(NCORES):
        m = dict(per_core[c])
        m["xh"] = xh
        m["wh"] = np.ascontiguousarray(whp)
        in_maps.append(m)
    return T, in_maps, token_ids


def _run(inputs, trace=False, trace_kwargs=None):
    T, in_maps, token_ids = _prep(inputs)
    nc = _build_nc(T)
    res = run_bass_kernel_spmd(
        nc, in_maps, core_ids=list(range(NCORES)), trace=trace,
        **(trace_kwargs or {}),
    )
    out = np.zeros((N_TOK, D_OUT), np.float32)
    for c in range(NCORES):
        oT = res.results[c]["outT"]            # [P, D_OUT//P, TPC]
        out[token_ids[c]] = oT.transpose(2, 1, 0).reshape(TPC, D_OUT)
    return out, res


def kernel(**inputs) -> np.ndarray:
    out, _ = _run(inputs, trace=bool(int(os.environ.get("KERNEL_TRACE", "0"))))
    return out


# revision 17
# speedup vs baseline: 1.2697x; 1.0133x over previous
# Trainium2 Bass kernel for nn_ExpertLinear (MoE grouped GEMM with routing).
#
# Strategy: data-parallel over tokens (8 cores), full weights replicated,
# fp16 compute with fp32 PSUM accumulation (measured rel err ~3.6e-4).
# Per core:
#   1. dma_gather(transpose=True) pulls the core's token rows from HBM x
#      (fp16) directly into the transposed [d_in, tokens] stationary-operand
#      layout, grouped by expert (per-expert groups padded to 128-row tiles,
#      tile counts shared across cores so one NEFF serves all 8).
#   2. Grouped GEMM per expert row-tile: 8 k-tile matmuls accumulate into
#      PSUM; eviction applies the per-row gate (DVE tensor_scalar) while
#      casting into an fp16 y buffer.
#   3. Combine: SBUF-source dma_gather (transpose mode) pulls each token's
#      two gated y rows (all gathers batched into dedicated tiles), DVE adds
#      produce out^T chunks written to DRAM. Host de-transposes and scatters
#      rows by the token->core assignment.
import os
import numpy as np

import concourse.bacc as bacc
import concourse.bass as bass
import concourse.mybir as mybir
import concourse.tile as tile
from concourse.bass_utils import run_bass_kernel_spmd

N_TOK = 8192
TOPK = 2
N_EXP = 8
D_IN = 1024
D_OUT = 1024
NCORES = 8
TPC = N_TOK // NCORES          # tokens per core
P = 128
KTILES = D_IN // P             # 8 k-tiles over d_in
F16 = mybir.dt.float16
F32 = mybir.dt.float32
I16 = mybir.dt.int16


def _pack16(flat):
    # [16, n/16] block (idx j at [j%16, j//16]), replicated into all eight
    # 16-partition groups — each GpSimd Q7 core reads its own copy.
    return np.ascontiguousarray(np.tile(flat.reshape(-1, 16).T, (8, 1)))


def _plan(tok, sei, g_row):
    """Host routing plan. Returns per-core index/gate tables and the shared
    per-expert tile counts T (max over cores, so one NEFF serves all 8)."""
    order_by_tok = np.argsort(tok, kind="stable")
    te = sei[order_by_tok].reshape(N_TOK, TOPK)
    pair_id = te[:, 0] * N_EXP + te[:, 1]

    # balanced token->core assignment: round-robin inside expert-pair groups
    order = np.argsort(pair_id, kind="stable")
    core_of_token = np.empty(N_TOK, np.int64)
    core_of_token[order] = np.arange(N_TOK) % NCORES

    token_ids = [np.where(core_of_token == c)[0] for c in range(NCORES)]
    token_pos = np.empty(N_TOK, np.int64)
    for c in range(NCORES):
        token_pos[token_ids[c]] = np.arange(TPC)

    core_of_row = core_of_token[tok]
    rows_per_core = [np.where(core_of_row == c)[0] for c in range(NCORES)]
    cnt = np.zeros((NCORES, N_EXP), np.int64)
    for c in range(NCORES):
        cnt[c] = np.bincount(sei[rows_per_core[c]], minlength=N_EXP)

    # process experts smallest-first (fast first gather)
    perm = np.argsort(cnt.max(axis=0), kind="stable")
    T = np.maximum(1, -(-cnt.max(axis=0)[perm] // P))
    off = np.concatenate([[0], np.cumsum(T) * P])
    NP = int(off[-1])
    slot_of_expert = np.empty(N_EXP, np.int64)
    slot_of_expert[perm] = np.arange(N_EXP)

    per_core = []
    for c in range(NCORES):
        rows_c = rows_per_core[c]
        s_c = slot_of_expert[sei[rows_c]]
        ordr = np.argsort(s_c, kind="stable")
        rows_c = rows_c[ordr]
        s_c = s_c[ordr]
        scnt = np.bincount(s_c, minlength=N_EXP)
        within = np.arange(rows_c.size) - np.concatenate(
            [[0], np.cumsum(scnt)])[s_c]
        loc = off[s_c] + within

        gidx_flat = np.zeros(NP, np.int16)
        grow_flat = np.zeros(NP, np.float32)
        gidx_flat[loc] = tok[rows_c].astype(np.int16)
        grow_flat[loc] = g_row[rows_c]

        pos = token_pos[tok[rows_c]]
        r0_flat = np.zeros(TPC, np.int16)
        r1_flat = np.zeros(TPC, np.int16)
        seen = np.zeros(TPC, bool)
        for j in range(rows_c.size):
            p_ = pos[j]
            if seen[p_]:
                r1_flat[p_] = loc[j]
            else:
                r0_flat[p_] = loc[j]
                seen[p_] = True
        assert seen.all()

        per_core.append(
            dict(
                gidx=_pack16(gidx_flat),
                grow=np.ascontiguousarray(grow_flat.reshape(-1, P).T),
                r0i=_pack16(r0_flat),
                r1i=_pack16(r1_flat),
            )
        )
    return T, NP, per_core, token_ids, perm


def _build_nc(T):
    NP = int(T.sum()) * P
    NB = NP // P
    off = np.concatenate([[0], np.cumsum(T)]) * P

    nc = bacc.Bacc("TRN2", target_bir_lowering=False, debug=False,
                   num_devices=NCORES)

    xh = nc.dram_tensor("xh", [N_TOK, D_IN], F16, kind="ExternalInput")
    wh = nc.dram_tensor("wh", [N_EXP, P, KTILES, D_OUT], F16,
                        kind="ExternalInput")
    gidx = nc.dram_tensor("gidx", [P, NP // 16], I16, kind="ExternalInput")
    grow = nc.dram_tensor("grow", [P, NB], F32, kind="ExternalInput")
    r0i = nc.dram_tensor("r0i", [P, TPC // 16], I16, kind="ExternalInput")
    r1i = nc.dram_tensor("r1i", [P, TPC // 16], I16, kind="ExternalInput")
    outT = nc.dram_tensor("outT", [P, D_OUT // P, TPC], F32,
                          kind="ExternalOutput")

    CH = 256
    NCH = TPC // CH

    # Pre-TileContext warmup: the first DMAGatherAnt triggers a ~15us Q7
    # extended-instruction library fetch. Issue a tiny dummy gather before
    # the Tile entry barrier so the fetch overlaps the preamble + input DMAs.
    warm_idx = nc.alloc_sbuf_tensor("warm_idx", [P, 8], I16)
    warm_dst = nc.alloc_sbuf_tensor("warm_dst", [P, P], F16)
    warm_sem = nc.alloc_semaphore("warm_set")
    warm_dma = nc.alloc_semaphore("warm_dma")
    nc.gpsimd.memset(warm_idx.ap(), 0).then_inc(warm_sem, 1)
    nc.gpsimd.wait_ge(warm_sem, 1)
    nc.gpsimd.dma_gather(
        warm_dst.ap().rearrange("p (a b) -> p a b", a=1),
        xh[:].rearrange("n (a b) -> (n a) b", b=P),
        warm_idx.ap(), num_idxs=P, num_idxs_reg=P, elem_size=P,
        transpose=True).then_inc(warm_dma, 16)
    nc.gpsimd.wait_ge(warm_dma, 16)

    with tile.TileContext(nc) as tc:
        with (
            tc.tile_pool(name="const", bufs=1) as kpool,
            tc.tile_pool(name="w", bufs=2) as wpool,
            tc.tile_pool(name="xT", bufs=4) as xpool,
            tc.tile_pool(name="y", bufs=1) as ypool,
            tc.tile_pool(name="cmb", bufs=1) as cpool,
            tc.tile_pool(name="ot", bufs=2) as opool,
            tc.tile_pool(name="ps", bufs=4, space="PSUM") as ppool,
        ):
            gidx_t = kpool.tile([P, NP // 16], I16)
            nc.gpsimd.dma_start(gidx_t[:], gidx[:])

            # dispatch gathers up front (expert 0 split per row-tile so the
            # first matmul starts early)
            x_tiles = []
            for e in range(N_EXP):
                ne = int(T[e]) * P
                if e == 0:
                    parts = []
                    for t in range(int(T[e])):
                        xp = xpool.tile([P, KTILES, P], F16, tag=f"x0_{t}")
                        nc.gpsimd.dma_gather(
                            xp[:], xh[:],
                            gidx_t[:, (off[e] + t * P) // 16:
                                   (off[e] + (t + 1) * P) // 16],
                            num_idxs=P, num_idxs_reg=P, elem_size=D_IN,
                            transpose=True,
                        )
                        parts.append(xp)
                    x_tiles.append(parts)
                else:
                    x_t = xpool.tile([P, KTILES, ne], F16, tag="xT")
                    nc.gpsimd.dma_gather(
                        x_t[:], xh[:],
                        gidx_t[:, off[e] // 16:(off[e] + ne) // 16],
                        num_idxs=ne, num_idxs_reg=ne, elem_size=D_IN,
                        transpose=True,
                    )
                    x_tiles.append(x_t)

            grow_t = kpool.tile([P, NB], F32)
            nc.sync.dma_start(grow_t[:], grow[:])
            r0_t = kpool.tile([P, TPC // 16], I16)
            nc.sync.dma_start(r0_t[:], r0i[:])
            r1_t = kpool.tile([P, TPC // 16], I16)
            nc.sync.dma_start(r1_t[:], r1i[:])

            y_t = ypool.tile([P, NB, D_OUT], F16)

            for e in range(N_EXP):
                w_t = wpool.tile([P, KTILES, D_OUT], F16, tag="w")
                for kk in range(KTILES):
                    nc.scalar.dma_start(w_t[:, kk], wh[e, :, kk])
                x_t = x_tiles[e]
                for t in range(int(T[e])):
                    rt_g = off[e] // P + t
                    ps0 = ppool.tile([P, 512], F32, tag="ps")
                    ps1 = ppool.tile([P, 512], F32, tag="ps")
                    for kk in range(KTILES):
                        if e == 0:
                            lhsT = x_t[t][:, kk, :]
                        else:
                            lhsT = x_t[:, kk, t * P:(t + 1) * P]
                        nc.tensor.matmul(ps0[:], lhsT, w_t[:, kk, 0:512],
                                         start=(kk == 0),
                                         stop=(kk == KTILES - 1))
                        nc.tensor.matmul(ps1[:], lhsT, w_t[:, kk, 512:1024],
                                         start=(kk == 0),
                                         stop=(kk == KTILES - 1))
                    gsc = grow_t[:, rt_g:rt_g + 1]
                    nc.vector.tensor_scalar_mul(y_t[:, rt_g, 0:512],
                                                ps0[:], gsc)
                    nc.vector.tensor_scalar_mul(y_t[:, rt_g, 512:1024],
                                                ps1[:], gsc)

            # combine: batch ALL gathers (dedicated tiles), then adds+stores
            gath = []
            for h in range(NCH):
                g0 = cpool.tile([P, D_OUT // P, CH], F16, tag=f"c0_{h}")
                g1 = cpool.tile([P, D_OUT // P, CH], F16, tag=f"c1_{h}")
                for dst, ridx in ((g0, r0_t), (g1, r1_t)):
                    nc.gpsimd.dma_gather(
                        dst[:], y_t[:],
                        ridx[:, h * (CH // 16):(h + 1) * (CH // 16)],
                        num_idxs=CH, num_idxs_reg=CH, elem_size=D_OUT,
                        transpose=True,
                        sbuf_tokens_per_rank=P,
                        sbuf_free_dim_per_rank=D_OUT * 2,
                    )
                gath.append((g0, g1))
            for h, (g0, g1) in enumerate(gath):
                ot = opool.tile([P, D_OUT // P, CH], F32, tag="ot")
                nc.vector.tensor_add(out=ot[:], in0=g0[:], in1=g1[:])
                nc.sync.dma_start(outT[:, :, h * CH:(h + 1) * CH], ot[:])

    nc.compile()
    return nc


def _prep(inputs):
    x = np.asarray(inputs["input"], np.float32)
    w = np.asarray(inputs["weight"], np.float32)
    k = int(np.asarray(inputs["k"]))
    assert k == TOPK
    sei = np.asarray(inputs["sorted_expert_indices"]).astype(np.int64)
    ssi = np.asarray(inputs["sorted_scattered_indices"]).astype(np.int64)
    gates = np.asarray(inputs["gates"], np.float32)

    tok = ssi // k
    g_row = gates.reshape(-1)[ssi]

    T, NP, per_core, token_ids, perm = _plan(tok, sei, g_row)

    xh = x.astype(np.float16)
    whp = np.ascontiguousarray(
        w.reshape(N_EXP, KTILES, P, D_OUT).transpose(0, 2, 1, 3)
    ).astype(np.float16)[perm]

    in_maps = []
    for c in range(NCORES):
        m = dict(per_core[c])
        m["xh"] = xh
        m["wh"] = np.ascontiguousarray(whp)
        in_maps.append(m)
    return T, in_maps, token_ids


def _run(inputs, trace=False, trace_kwargs=None):
    T, in_maps, token_ids = _prep(inputs)
    nc = _build_nc(T)
    res = run_bass_kernel_spmd(
        nc, in_maps, core_ids=list(range(NCORES)), trace=trace,
        **(trace_kwargs or {}),
    )
    out = np.zeros((N_TOK, D_OUT), np.float32)
    for c in range(NCORES):
        oT = res.results[c]["outT"]            # [P, D_OUT//P, TPC]
        out[token_ids[c]] = oT.transpose(2, 1, 0).reshape(TPC, D_OUT)
    return out, res


def kernel(**inputs) -> np.ndarray:
    out, _ = _run(inputs, trace=bool(int(os.environ.get("KERNEL_TRACE", "0"))))
    return out
